# revision 3
# baseline (speedup 1.0000x reference)
"""CoxTime loss kernel for 8 Trainium2 NeuronCores.

Strategy (data-parallel over B):
  Each core reduces its (32768, 128) f32 logits shard directly to the
  risk-set masked column sums
      sumexp[k] = sum_j 1{label_j >= k} * exp(logits[j, k])
  via three single-pass elementwise stages, one per engine
      scalar:  E  = exp(x)            (f32 -> bf16)
      vector:  m  = (iota_k <= label) (bf16 compare)
      gpsimd:  mE = m * E             (bf16)
  and a TensorEngine matmul with a ones[128,1] stationary vector
  (loaded once) that sums mE across partitions, accumulating in PSUM
  over row-tiles.  The host all-reduces the 8 per-core vectors and
  finishes with the O(B) 1-D work: numer/n_ev from labels+events+own
  logits, the log, and the scalar reduction.
"""

import numpy as np

import concourse.bacc as bacc
import concourse.bass as bass
import concourse.mybir as mybir
import concourse.tile as tile
from concourse.bass_utils import run_bass_kernel_spmd

B = 262144
K = 128
NCORES = 8
BC = B // NCORES  # rows per core
P = 128           # partitions
NT = BC // P      # row-tiles per core (column index t in the [P, NT] view)
TPB = 16          # row-tiles per DMA'd big tile
NBIG = NT // TPB  # big tiles per core
NBANK = 4         # PSUM banks; each holds [1, 4*K] partial sums

f32 = mybir.dt.float32
bf16 = mybir.dt.bfloat16
i32 = mybir.dt.int32

LAST_EXEC_NS = None
LAST_TRACE = None
LAST_PROFILE_JSON = None


def build_nc():
    """Build the per-core Bass program."""
    nc = bacc.Bacc("TRN2", target_bir_lowering=False)
    logits = nc.declare_dram_parameter("logits", [BC, K], f32, isOutput=False)
    labcols = nc.declare_dram_parameter("labcols", [P, NT], f32, isOutput=False)
    out = nc.declare_dram_parameter("out", [1, TPB * K], f32, isOutput=True)

    FW = TPB * K          # free width of a big tile
    BW = FW // NBANK      # columns per PSUM bank

    with tile.TileContext(nc) as tc:
        with (
            tc.tile_pool(name="const", bufs=1) as cpool,
            tc.tile_pool(name="lt", bufs=3) as ltpool,
            tc.tile_pool(name="ee", bufs=3) as epool,
            tc.tile_pool(name="mm", bufs=3) as mpool,
            tc.tile_pool(name="me", bufs=3) as mepool,
            tc.tile_pool(name="psum", bufs=1, space="PSUM") as pspool,
        ):
            labc_f = cpool.tile([P, NT], f32)
            nc.sync.dma_start(out=labc_f[:], in_=labcols.ap())
            labc = cpool.tile([P, NT], bf16)
            nc.vector.tensor_copy(labc[:], labc_f[:])

            ones = cpool.tile([P, 1], bf16)
            nc.gpsimd.memset(ones[:], 1.0)

            # iota over the k axis, replicated TPB times along free dim
            iota_i = cpool.tile([P, FW], i32)
            nc.gpsimd.iota(iota_i[:], pattern=[[0, TPB], [1, K]], base=0,
                           channel_multiplier=0)
            iota_f = cpool.tile([P, FW], bf16)
            nc.vector.tensor_copy(iota_f[:], iota_i[:])
            iota_f3 = iota_f[:].rearrange("p (q k) -> p q k", k=K)

            psums = [pspool.tile([1, BW], f32, name=f"ps{b}", tag=f"ps{b}")
                     for b in range(NBANK)]

            # row j = p*NT + g*TPB + q  ->  per-partition contiguous 8KB lines
            lg3 = logits.ap().rearrange("(p g q) k -> g p (q k)", p=P, q=TPB)

            for g in range(NBIG):
                lt = ltpool.tile([P, FW], f32)
                nc.sync.dma_start(out=lt[:], in_=lg3[g])
                lt3 = lt[:].rearrange("p (q k) -> p q k", k=K)

                # E = exp(logits), cast to bf16
                ee = epool.tile([P, FW], bf16)
                ee3 = ee[:].rearrange("p (q k) -> p q k", k=K)
                nc.scalar.activation(out=ee3, in_=lt3,
                                     func=mybir.ActivationFunctionType.Exp)

                # m[p, q, k] = (k <= label[p, g*TPB+q])  == (label >= k)
                mm = mpool.tile([P, FW], bf16)
                mm3 = mm[:].rearrange("p (q k) -> p q k", k=K)
                lab_b = labc[:, g * TPB:(g + 1) * TPB][:, :, None].to_broadcast(
                    [P, TPB, K])
                nc.vector.tensor_tensor(out=mm3, in0=iota_f3, in1=lab_b,
                                        op=mybir.AluOpType.is_le)

                # mE = m * E
                me = mepool.tile([P, FW], bf16)
                nc.gpsimd.tensor_tensor(out=me[:], in0=ee[:], in1=mm[:],
                                        op=mybir.AluOpType.mult)

                # column sums across partitions, accumulated over g in PSUM
                for b in range(NBANK):
                    nc.tensor.matmul(
                        out=psums[b][:],
                        lhsT=ones[:],
                        rhs=me[:, b * BW:(b + 1) * BW],
                        start=(g == 0),
                        stop=(g == NBIG - 1),
                    )

            osb = cpool.tile([1, FW], f32)
            for b in range(NBANK):
                nc.vector.tensor_copy(osb[:, b * BW:(b + 1) * BW],
                                      psums[b][:])
            nc.sync.dma_start(out=out.ap(), in_=osb[:])

    nc.compile()
    return nc


def _shard_inputs(logits, labels):
    """Build the 8 per-core input maps (host-side layout only)."""
    logits = np.ascontiguousarray(np.asarray(logits, dtype=np.float32))
    labels = np.asarray(labels, dtype=np.int32)
    in_maps = []
    for i in range(NCORES):
        sl = slice(i * BC, (i + 1) * BC)
        lab = labels[sl].astype(np.float32).reshape(P, NT)
        in_maps.append({
            "logits": logits[sl],
            "labcols": np.ascontiguousarray(lab),
        })
    return in_maps


def _finish(outs, logits, labels, events):
    """Host epilogue: all-reduce sumexp, numer/n_ev from 1-D data, log,
    and the final scalar reduction."""
    labels = np.asarray(labels, dtype=np.int32)
    events = np.asarray(events, dtype=np.int32)
    sumexp = np.zeros(K, dtype=np.float64)
    for o in outs:
        sumexp += o.astype(np.float64).reshape(TPB, K).sum(axis=0)
    ev = events == 1
    own = np.asarray(logits)[np.arange(labels.shape[0]), labels].astype(
        np.float64)
    n_ev = np.bincount(labels[ev], minlength=K).astype(np.float64)
    numer = np.bincount(labels[ev], weights=own[ev], minlength=K)
    with np.errstate(divide="ignore"):
        denom_log = np.log(sumexp)
    terms = np.where(n_ev > 0, numer - n_ev * denom_log, 0.0)
    n_total = max(n_ev.sum(), 1.0)
    return np.array(-terms.sum() / n_total, dtype=np.float32)


def kernel(logits, labels, events, _trace=False):
    global LAST_EXEC_NS, LAST_TRACE, LAST_PROFILE_JSON
    in_maps = _shard_inputs(logits, labels)
    nc = build_nc()
    try:
        res = run_bass_kernel_spmd(nc, in_maps, core_ids=list(range(NCORES)),
                                   trace=_trace)
    except Exception:
        # one retry: absorbs transient NRT device-unrecoverable hiccups
        res = run_bass_kernel_spmd(nc, in_maps, core_ids=list(range(NCORES)),
                                   trace=_trace)
    LAST_EXEC_NS = res.exec_time_ns
    LAST_TRACE = res.instructions_and_trace
    LAST_PROFILE_JSON = res.profile_json
    outs = [res.results[i]["out"] for i in range(NCORES)]
    return _finish(outs, logits, labels, events)


# revision 4
# speedup vs baseline: 1.6072x; 1.6072x over previous
"""CoxTime loss kernel for 8 Trainium2 NeuronCores.

Strategy (data-parallel over B):
  Each core reduces its (32768, 128) f32 logits shard to a (128, 128)
  binned summary using the TensorEngine with an on-the-fly one-hot of
  labels:
      S[c, k] = sum_{j: label_j == c} exp(logits[j, k])
  Per big tile only two full-rate elementwise passes are needed —
  exp on the scalar engine and the one-hot is_equal on the vector
  engine (1x DVE mode; the broadcast label operand rules out 2x).
  The label mask (labels >= k) is applied by the matmul binning plus
  a host-side triangular sum, so no mask/multiply passes are needed.
  The host all-reduces the 8 summaries and finishes with the O(B) 1-D
  work: numer/n_ev from labels+events+own logits, the log, and the
  scalar reduction.
"""

import numpy as np

import concourse.bacc as bacc
import concourse.bass as bass
import concourse.mybir as mybir
import concourse.tile as tile
from concourse.bass_utils import run_bass_kernel_spmd

B = 262144
K = 128
NCORES = 8
BC = B // NCORES  # rows per core
P = 128           # partitions
NT = BC // P      # row-tiles per core (column index t in the [P, NT] view)
TPB = 32          # row-tiles per DMA'd big tile
NBIG = NT // TPB  # big tiles per core
NBANK = 8         # PSUM banks rotated across row-tiles for matmul ILP

f32 = mybir.dt.float32
bf16 = mybir.dt.bfloat16
i32 = mybir.dt.int32

LAST_EXEC_NS = None
LAST_TRACE = None
LAST_PROFILE_JSON = None


def build_nc():
    """Build the per-core Bass program."""
    nc = bacc.Bacc("TRN2", target_bir_lowering=False)
    logits = nc.declare_dram_parameter("logits", [BC, K], f32, isOutput=False)
    labcols = nc.declare_dram_parameter("labcols", [P, NT], f32, isOutput=False)
    out = nc.declare_dram_parameter("out", [P, NBANK * K], f32, isOutput=True)

    FW = TPB * K  # free width of a big tile

    with tile.TileContext(nc) as tc:
        with (
            tc.tile_pool(name="const", bufs=1) as cpool,
            tc.tile_pool(name="lt", bufs=3) as ltpool,
            tc.tile_pool(name="ee", bufs=3) as epool,
            tc.tile_pool(name="oh", bufs=3) as ohpool,
            tc.tile_pool(name="psum", bufs=1, space="PSUM") as pspool,
        ):
            labc_f = cpool.tile([P, NT], f32)
            nc.sync.dma_start(out=labc_f[:], in_=labcols.ap())
            labc = cpool.tile([P, NT], bf16)
            nc.vector.tensor_copy(labc[:], labc_f[:])

            # iota over the k axis, replicated TPB times along free dim
            iota_i = cpool.tile([P, FW], i32)
            nc.gpsimd.iota(iota_i[:], pattern=[[0, TPB], [1, K]], base=0,
                           channel_multiplier=0)
            iota_f = cpool.tile([P, FW], bf16)
            nc.vector.tensor_copy(iota_f[:], iota_i[:])
            iota_f3 = iota_f[:].rearrange("p (q k) -> p q k", k=K)

            psums = [pspool.tile([P, K], f32, name=f"ps{b}", tag=f"ps{b}")
                     for b in range(NBANK)]

            # row j = p*NT + g*TPB + q  ->  per-partition contiguous 16KB lines
            lg3 = logits.ap().rearrange("(p g q) k -> g p (q k)", p=P, q=TPB)

            for g in range(NBIG):
                lt = ltpool.tile([P, FW], f32)
                nc.sync.dma_start(out=lt[:], in_=lg3[g])
                lt3 = lt[:].rearrange("p (q k) -> p q k", k=K)

                # E = exp(logits), cast to bf16
                ee = epool.tile([P, FW], bf16)
                ee3 = ee[:].rearrange("p (q k) -> p q k", k=K)
                nc.scalar.activation(out=ee3, in_=lt3,
                                     func=mybir.ActivationFunctionType.Exp)

                # one-hot of labels: oh[p, q, k] = (label[p, g*TPB+q] == k)
                oh = ohpool.tile([P, FW], bf16)
                oh3 = oh[:].rearrange("p (q k) -> p q k", k=K)
                lab_b = labc[:, g * TPB:(g + 1) * TPB][:, :, None].to_broadcast(
                    [P, TPB, K])
                nc.vector.tensor_tensor(out=oh3, in0=iota_f3, in1=lab_b,
                                        op=mybir.AluOpType.is_equal)

                # bin exp values by label: S[c, k] += sum_p oh[p,c] * E[p,k]
                for q in range(TPB):
                    t = g * TPB + q
                    b = t % NBANK
                    nc.tensor.matmul(
                        out=psums[b][:],
                        lhsT=oh[:, q * K:(q + 1) * K],
                        rhs=ee[:, q * K:(q + 1) * K],
                        start=(t < NBANK),
                        stop=(t >= NT - NBANK),
                    )

            osb = cpool.tile([P, NBANK * K], f32)
            for b in range(NBANK):
                nc.vector.tensor_copy(osb[:, b * K:(b + 1) * K], psums[b][:])
            nc.sync.dma_start(out=out.ap(), in_=osb[:])

    nc.compile()
    return nc


def _shard_inputs(logits, labels):
    """Build the 8 per-core input maps (host-side layout only)."""
    logits = np.ascontiguousarray(np.asarray(logits, dtype=np.float32))
    labels = np.asarray(labels, dtype=np.int32)
    in_maps = []
    for i in range(NCORES):
        sl = slice(i * BC, (i + 1) * BC)
        lab = labels[sl].astype(np.float32).reshape(P, NT)
        in_maps.append({
            "logits": logits[sl],
            "labcols": np.ascontiguousarray(lab),
        })
    return in_maps


def _finish(outs, logits, labels, events):
    """Host epilogue: all-reduce binned sums, triangular sum, numer/n_ev
    from 1-D data, the log, and the final scalar reduction."""
    labels = np.asarray(labels, dtype=np.int32)
    events = np.asarray(events, dtype=np.int32)
    S = np.zeros((P, K), dtype=np.float64)
    for o in outs:
        S += o.astype(np.float64).reshape(P, NBANK, K).sum(axis=1)
    # sumexp[k] = sum over label bins c >= k
    sumexp = (S * np.tri(K)).sum(axis=0)
    ev = events == 1
    own = np.asarray(logits)[np.arange(labels.shape[0]), labels].astype(
        np.float64)
    n_ev = np.bincount(labels[ev], minlength=K).astype(np.float64)
    numer = np.bincount(labels[ev], weights=own[ev], minlength=K)
    with np.errstate(divide="ignore"):
        denom_log = np.log(sumexp)
    terms = np.where(n_ev > 0, numer - n_ev * denom_log, 0.0)
    n_total = max(n_ev.sum(), 1.0)
    return np.array(-terms.sum() / n_total, dtype=np.float32)


def kernel(logits, labels, events, _trace=False):
    global LAST_EXEC_NS, LAST_TRACE, LAST_PROFILE_JSON
    in_maps = _shard_inputs(logits, labels)
    nc = build_nc()
    try:
        res = run_bass_kernel_spmd(nc, in_maps, core_ids=list(range(NCORES)),
                                   trace=_trace)
    except Exception:
        # one retry: absorbs transient NRT device-unrecoverable hiccups
        res = run_bass_kernel_spmd(nc, in_maps, core_ids=list(range(NCORES)),
                                   trace=_trace)
    LAST_EXEC_NS = res.exec_time_ns
    LAST_TRACE = res.instructions_and_trace
    LAST_PROFILE_JSON = res.profile_json
    outs = [res.results[i]["out"] for i in range(NCORES)]
    return _finish(outs, logits, labels, events)


# revision 6
# speedup vs baseline: 1.6289x; 1.0135x over previous
"""CoxTime loss kernel for 8 Trainium2 NeuronCores.

Strategy (data-parallel over B):
  Each core reduces its (32768, 128) f32 logits shard to a (128, 128)
  binned summary using the TensorEngine with an on-the-fly one-hot of
  labels:
      S[c, k] = sum_{j: label_j == c} exp(logits[j, k])
  Per big tile only two full-rate elementwise passes are needed —
  exp on the scalar engine and the one-hot is_equal on the vector
  engine (1x DVE mode; the broadcast label operand rules out 2x).
  The label mask (labels >= k) is applied by the matmul binning plus
  a host-side triangular sum, so no mask/multiply passes are needed.
  The iota row and the labels are DMA'd in as bf16 constants so no
  on-device iota/cast sits on the critical path.  The host all-reduces
  the 8 summaries and finishes with the O(B) 1-D work: numer/n_ev from
  labels+events+own logits, the log, and the scalar reduction.
"""

import ml_dtypes
import numpy as np

import concourse.bacc as bacc
import concourse.bass as bass
import concourse.mybir as mybir
import concourse.tile as tile
from concourse.bass_utils import run_bass_kernel_spmd

B = 262144
K = 128
NCORES = 8
BC = B // NCORES  # rows per core
P = 128           # partitions
NT = BC // P      # row-tiles per core (column index t in the [P, NT] view)
TPB = 16          # row-tiles per DMA'd big tile
NBIG = NT // TPB  # big tiles per core
NBANK = 8         # PSUM banks rotated across row-tiles for matmul ILP

f32 = mybir.dt.float32
bf16 = mybir.dt.bfloat16

LAST_EXEC_NS = None
LAST_TRACE = None
LAST_PROFILE_JSON = None


def build_nc():
    """Build the per-core Bass program."""
    nc = bacc.Bacc("TRN2", target_bir_lowering=False)
    logits = nc.declare_dram_parameter("logits", [BC, K], f32, isOutput=False)
    labcols = nc.declare_dram_parameter("labcols", [P, NT], bf16,
                                        isOutput=False)
    iotak = nc.declare_dram_parameter("iotak", [P, K], bf16, isOutput=False)
    out = nc.declare_dram_parameter("out", [P, NBANK * K], f32, isOutput=True)

    FW = TPB * K  # free width of a big tile

    with tile.TileContext(nc) as tc:
        with (
            tc.tile_pool(name="const", bufs=1) as cpool,
            tc.tile_pool(name="lt", bufs=3) as ltpool,
            tc.tile_pool(name="ee", bufs=6) as epool,
            tc.tile_pool(name="oh", bufs=6) as ohpool,
            tc.tile_pool(name="psum", bufs=1, space="PSUM") as pspool,
        ):
            labc = cpool.tile([P, NT], bf16)
            nc.sync.dma_start(out=labc[:], in_=labcols.ap())
            iota = cpool.tile([P, K], bf16)
            nc.sync.dma_start(out=iota[:], in_=iotak.ap())
            iota_b = iota[:][:, None, :].to_broadcast([P, TPB, K])

            psums = [pspool.tile([P, K], f32, name=f"ps{b}", tag=f"ps{b}")
                     for b in range(NBANK)]

            # row j = p*NT + g*TPB + q  ->  per-partition contiguous 8KB lines
            lg3 = logits.ap().rearrange("(p g q) k -> g p (q k)", p=P, q=TPB)

            for g in range(NBIG):
                lt = ltpool.tile([P, FW], f32)
                nc.sync.dma_start(out=lt[:], in_=lg3[g])
                lt3 = lt[:].rearrange("p (q k) -> p q k", k=K)

                # E = exp(logits), cast to bf16
                ee = epool.tile([P, FW], bf16)
                ee3 = ee[:].rearrange("p (q k) -> p q k", k=K)
                nc.scalar.activation(out=ee3, in_=lt3,
                                     func=mybir.ActivationFunctionType.Exp)

                # one-hot of labels: oh[p, q, k] = (label[p, g*TPB+q] == k)
                oh = ohpool.tile([P, FW], bf16)
                oh3 = oh[:].rearrange("p (q k) -> p q k", k=K)
                lab_b = labc[:, g * TPB:(g + 1) * TPB][:, :, None].to_broadcast(
                    [P, TPB, K])
                nc.vector.tensor_tensor(out=oh3, in0=iota_b, in1=lab_b,
                                        op=mybir.AluOpType.is_equal)

                # bin exp values by label: S[c, k] += sum_p oh[p,c] * E[p,k]
                for q in range(TPB):
                    t = g * TPB + q
                    b = t % NBANK
                    nc.tensor.matmul(
                        out=psums[b][:],
                        lhsT=oh[:, q * K:(q + 1) * K],
                        rhs=ee[:, q * K:(q + 1) * K],
                        start=(t < NBANK),
                        stop=(t >= NT - NBANK),
                    )

            osb = cpool.tile([P, NBANK * K], f32)
            for b in range(NBANK):
                eng = nc.vector if b % 2 == 0 else nc.scalar
                if eng is nc.vector:
                    eng.tensor_copy(osb[:, b * K:(b + 1) * K], psums[b][:])
                else:
                    eng.copy(osb[:, b * K:(b + 1) * K], psums[b][:])
            nc.sync.dma_start(out=out.ap(), in_=osb[:])

    nc.compile()
    return nc


def _shard_inputs(logits, labels):
    """Build the 8 per-core input maps (host-side layout only)."""
    logits = np.ascontiguousarray(np.asarray(logits, dtype=np.float32))
    labels = np.asarray(labels, dtype=np.int32)
    iota = np.broadcast_to(np.arange(K, dtype=np.float32), (P, K)).astype(
        ml_dtypes.bfloat16)
    in_maps = []
    for i in range(NCORES):
        sl = slice(i * BC, (i + 1) * BC)
        lab = labels[sl].astype(np.float32).reshape(P, NT).astype(
            ml_dtypes.bfloat16)
        in_maps.append({
            "logits": logits[sl],
            "labcols": np.ascontiguousarray(lab),
            "iotak": iota,
        })
    return in_maps


def _finish(outs, logits, labels, events):
    """Host epilogue: all-reduce binned sums, triangular sum, numer/n_ev
    from 1-D data, the log, and the final scalar reduction."""
    labels = np.asarray(labels, dtype=np.int32)
    events = np.asarray(events, dtype=np.int32)
    S = np.zeros((P, K), dtype=np.float64)
    for o in outs:
        S += o.astype(np.float64).reshape(P, NBANK, K).sum(axis=1)
    # sumexp[k] = sum over label bins c >= k
    sumexp = (S * np.tri(K)).sum(axis=0)
    ev = events == 1
    own = np.asarray(logits)[np.arange(labels.shape[0]), labels].astype(
        np.float64)
    n_ev = np.bincount(labels[ev], minlength=K).astype(np.float64)
    numer = np.bincount(labels[ev], weights=own[ev], minlength=K)
    with np.errstate(divide="ignore"):
        denom_log = np.log(sumexp)
    terms = np.where(n_ev > 0, numer - n_ev * denom_log, 0.0)
    n_total = max(n_ev.sum(), 1.0)
    return np.array(-terms.sum() / n_total, dtype=np.float32)


def kernel(logits, labels, events, _trace=False):
    global LAST_EXEC_NS, LAST_TRACE, LAST_PROFILE_JSON
    in_maps = _shard_inputs(logits, labels)
    nc = build_nc()
    try:
        res = run_bass_kernel_spmd(nc, in_maps, core_ids=list(range(NCORES)),
                                   trace=_trace)
    except Exception:
        # one retry: absorbs transient NRT device-unrecoverable hiccups
        res = run_bass_kernel_spmd(nc, in_maps, core_ids=list(range(NCORES)),
                                   trace=_trace)
    LAST_EXEC_NS = res.exec_time_ns
    LAST_TRACE = res.instructions_and_trace
    LAST_PROFILE_JSON = res.profile_json
    outs = [res.results[i]["out"] for i in range(NCORES)]
    return _finish(outs, logits, labels, events)


# revision 8
# speedup vs baseline: 1.7235x; 1.0581x over previous
"""CoxTime loss kernel for 8 Trainium2 NeuronCores.

Strategy (data-parallel over B):
  Each core reduces its (32768, 128) f32 logits shard to a (128, 128)
  binned summary using the TensorEngine with an on-the-fly one-hot of
  labels:
      S[c, k] = sum_{j: label_j == c} exp(logits[j, k])
  Layout [P, NT] (row j = p*NT + t) keeps every DMA partition-line
  contiguous AND gives per-partition label columns, so the one-hot is
  a single-source tensor_scalar(is_equal) per row-tile — the DVE's 4x
  mode — instead of a broadcast tensor_tensor (1x).  exp runs on the
  scalar engine.  The label mask (labels >= k) is applied by the
  matmul binning plus a host-side triangular sum.  The iota row and
  labels are DMA'd in as bf16 constants.  The host all-reduces the 8
  summaries and finishes with the O(B) 1-D work: numer/n_ev from
  labels+events+own logits, the log, and the scalar reduction.
"""

import ml_dtypes
import numpy as np

import concourse.bacc as bacc
import concourse.bass as bass
import concourse.mybir as mybir
import concourse.tile as tile
from concourse.bass_utils import run_bass_kernel_spmd

B = 262144
K = 128
NCORES = 8
BC = B // NCORES  # rows per core
P = 128           # partitions
NT = BC // P      # row-tiles per core (column index t in the [P, NT] view)
NBANK = 8         # PSUM banks rotated across row-tiles for matmul ILP

# big-tile segments (start row-tile, row-tiles); tapered tail so the
# last exp/matmul chains drain quickly after the final DMA lands
SEGS = [(t, 16) for t in range(0, 240, 16)] + [(240, 8), (248, 8)]

f32 = mybir.dt.float32
bf16 = mybir.dt.bfloat16

LAST_EXEC_NS = None
LAST_TRACE = None
LAST_PROFILE_JSON = None


def build_nc():
    """Build the per-core Bass program."""
    nc = bacc.Bacc("TRN2", target_bir_lowering=False)
    logits = nc.declare_dram_parameter("logits", [BC, K], f32, isOutput=False)
    labcols = nc.declare_dram_parameter("labcols", [P, NT], f32,
                                        isOutput=False)
    iotak = nc.declare_dram_parameter("iotak", [P, K], bf16, isOutput=False)
    out = nc.declare_dram_parameter("out", [P, NBANK * K], f32, isOutput=True)

    with tile.TileContext(nc) as tc:
        with (
            tc.tile_pool(name="const", bufs=1) as cpool,
            tc.tile_pool(name="lt", bufs=6) as ltpool,
            tc.tile_pool(name="ee", bufs=4) as epool,
            tc.tile_pool(name="oh", bufs=4) as ohpool,
            tc.tile_pool(name="psum", bufs=1, space="PSUM") as pspool,
        ):
            labc = cpool.tile([P, NT], f32)
            nc.sync.dma_start(out=labc[:], in_=labcols.ap())
            iota = cpool.tile([P, K], bf16)
            nc.sync.dma_start(out=iota[:], in_=iotak.ap())

            psums = [pspool.tile([P, K], f32, name=f"ps{b}", tag=f"ps{b}")
                     for b in range(NBANK)]

            # row j = p*NT + t  ->  [P, t, k]; per-partition lines contiguous
            lg3 = logits.ap().rearrange("(p t) k -> p t k", p=P)

            for t0, tpb in SEGS:
                fw = tpb * K
                lt = ltpool.tile([P, fw], f32)
                nc.sync.dma_start(out=lt[:], in_=lg3[:, t0:t0 + tpb, :])
                lt3 = lt[:].rearrange("p (q k) -> p q k", k=K)

                # E = exp(logits), cast to bf16
                ee = epool.tile([P, fw], bf16)
                ee3 = ee[:].rearrange("p (q k) -> p q k", k=K)
                nc.scalar.activation(out=ee3, in_=lt3,
                                     func=mybir.ActivationFunctionType.Exp)

                # one-hot per row-tile: oh[p, k] = (label[p, t] == k)
                # (single-source tensor_scalar -> 4x DVE mode)
                oh = ohpool.tile([P, fw], bf16)
                for q in range(tpb):
                    t = t0 + q
                    nc.vector.tensor_scalar(
                        out=oh[:, q * K:(q + 1) * K], in0=iota[:],
                        scalar1=labc[:, t:t + 1], scalar2=None,
                        op0=mybir.AluOpType.is_equal)

                # bin exp values by label: S[c, k] += sum_p oh[p,c] * E[p,k]
                for q in range(tpb):
                    t = t0 + q
                    b = t % NBANK
                    nc.tensor.matmul(
                        out=psums[b][:],
                        lhsT=oh[:, q * K:(q + 1) * K],
                        rhs=ee[:, q * K:(q + 1) * K],
                        start=(t < NBANK),
                        stop=(t >= NT - NBANK),
                    )

            osb = cpool.tile([P, NBANK * K], f32)
            for b in range(NBANK):
                if b % 2 == 0:
                    nc.vector.tensor_copy(osb[:, b * K:(b + 1) * K],
                                          psums[b][:])
                else:
                    nc.scalar.copy(osb[:, b * K:(b + 1) * K], psums[b][:])
            nc.sync.dma_start(out=out.ap(), in_=osb[:])

    nc.compile()
    return nc


def _shard_inputs(logits, labels):
    """Build the 8 per-core input maps (host-side layout only)."""
    logits = np.ascontiguousarray(np.asarray(logits, dtype=np.float32))
    labels = np.asarray(labels, dtype=np.int32)
    iota = np.broadcast_to(np.arange(K, dtype=np.float32), (P, K)).astype(
        ml_dtypes.bfloat16)
    in_maps = []
    for i in range(NCORES):
        sl = slice(i * BC, (i + 1) * BC)
        lab = labels[sl].astype(np.float32).reshape(P, NT)
        in_maps.append({
            "logits": logits[sl],
            "labcols": np.ascontiguousarray(lab),
            "iotak": iota,
        })
    return in_maps


def _finish(outs, logits, labels, events):
    """Host epilogue: all-reduce binned sums, triangular sum, numer/n_ev
    from 1-D data, the log, and the final scalar reduction."""
    labels = np.asarray(labels, dtype=np.int32)
    events = np.asarray(events, dtype=np.int32)
    S = np.zeros((P, K), dtype=np.float64)
    for o in outs:
        S += o.astype(np.float64).reshape(P, NBANK, K).sum(axis=1)
    # sumexp[k] = sum over label bins c >= k
    sumexp = (S * np.tri(K)).sum(axis=0)
    ev = events == 1
    own = np.asarray(logits)[np.arange(labels.shape[0]), labels].astype(
        np.float64)
    n_ev = np.bincount(labels[ev], minlength=K).astype(np.float64)
    numer = np.bincount(labels[ev], weights=own[ev], minlength=K)
    with np.errstate(divide="ignore"):
        denom_log = np.log(sumexp)
    terms = np.where(n_ev > 0, numer - n_ev * denom_log, 0.0)
    n_total = max(n_ev.sum(), 1.0)
    return np.array(-terms.sum() / n_total, dtype=np.float32)


def kernel(logits, labels, events, _trace=False):
    global LAST_EXEC_NS, LAST_TRACE, LAST_PROFILE_JSON
    in_maps = _shard_inputs(logits, labels)
    nc = build_nc()
    try:
        res = run_bass_kernel_spmd(nc, in_maps, core_ids=list(range(NCORES)),
                                   trace=_trace)
    except Exception:
        # one retry: absorbs transient NRT device-unrecoverable hiccups
        res = run_bass_kernel_spmd(nc, in_maps, core_ids=list(range(NCORES)),
                                   trace=_trace)
    LAST_EXEC_NS = res.exec_time_ns
    LAST_TRACE = res.instructions_and_trace
    LAST_PROFILE_JSON = res.profile_json
    outs = [res.results[i]["out"] for i in range(NCORES)]
    return _finish(outs, logits, labels, events)


# revision 11
# speedup vs baseline: 1.7464x; 1.0133x over previous
"""CoxTime loss kernel for 8 Trainium2 NeuronCores.

Strategy (data-parallel over B):
  Each core reduces its (32768, 128) f32 logits shard to a (128, 128)
  binned summary using the TensorEngine with an on-the-fly one-hot of
  labels:
      S[c, k] = sum_{j: label_j == c} exp(logits[j, k])
  Layout [P, NT] (row j = p*NT + t) keeps every DMA partition-line
  contiguous AND gives per-partition label columns, so the one-hot is
  a single-source tensor_scalar(is_equal) per row-tile — the DVE's 4x
  mode — instead of a broadcast tensor_tensor (1x).  exp runs on the
  scalar engine.  The label mask (labels >= k) is applied by the
  matmul binning plus a host-side triangular sum.  The iota row and
  labels are DMA'd in as bf16 constants.  The host all-reduces the 8
  summaries and finishes with the O(B) 1-D work: numer/n_ev from
  labels+events+own logits, the log, and the scalar reduction.
"""

import ml_dtypes
import numpy as np

import concourse.bacc as bacc
import concourse.bass as bass
import concourse.mybir as mybir
import concourse.tile as tile
from concourse.bass_utils import run_bass_kernel_spmd

B = 262144
K = 128
NCORES = 8
BC = B // NCORES  # rows per core
P = 128           # partitions
NT = BC // P      # row-tiles per core (column index t in the [P, NT] view)
NBANK = 8         # PSUM banks rotated across row-tiles for matmul ILP

TPB = 16          # row-tiles per DMA'd big tile
HPB = TPB // 2    # row-tiles per exp/one-hot chunk (finer pipeline grain)

f32 = mybir.dt.float32
bf16 = mybir.dt.bfloat16

LAST_EXEC_NS = None
LAST_TRACE = None
LAST_PROFILE_JSON = None


def build_nc():
    """Build the per-core Bass program."""
    nc = bacc.Bacc("TRN2", target_bir_lowering=False)
    logits = nc.declare_dram_parameter("logits", [BC, K], f32, isOutput=False)
    labcols = nc.declare_dram_parameter("labcols", [P, NT], f32,
                                        isOutput=False)
    iotak = nc.declare_dram_parameter("iotak", [P, K], bf16, isOutput=False)
    out = nc.declare_dram_parameter("out", [P, NBANK * K], f32, isOutput=True)

    with tile.TileContext(nc) as tc:
        with (
            tc.tile_pool(name="const", bufs=1) as cpool,
            tc.tile_pool(name="lt", bufs=6) as ltpool,
            tc.tile_pool(name="ee", bufs=8) as epool,
            tc.tile_pool(name="oh", bufs=8) as ohpool,
            tc.tile_pool(name="psum", bufs=1, space="PSUM") as pspool,
        ):
            labc = cpool.tile([P, NT], f32)
            nc.sync.dma_start(out=labc[:], in_=labcols.ap())
            iota = cpool.tile([P, K], bf16)
            nc.sync.dma_start(out=iota[:], in_=iotak.ap())

            psums = [pspool.tile([P, K], f32, name=f"ps{b}", tag=f"ps{b}")
                     for b in range(NBANK)]

            # row j = p*NT + t  ->  [P, t, k]; per-partition lines contiguous
            lg3 = logits.ap().rearrange("(p t) k -> p t k", p=P)

            HW = HPB * K
            for g in range(NT // TPB):
                t0 = g * TPB
                lt = ltpool.tile([P, TPB * K], f32)
                nc.sync.dma_start(out=lt[:], in_=lg3[:, t0:t0 + TPB, :])

                # half-seg chunks: finer exp grain keeps the matmul
                # stream right behind the DMA instead of backlogging
                for h in range(2):
                    h0 = t0 + h * HPB

                    # E = exp(logits), cast to bf16
                    ee = epool.tile([P, HW], bf16)
                    nc.scalar.activation(
                        out=ee[:], in_=lt[:, h * HW:(h + 1) * HW],
                        func=mybir.ActivationFunctionType.Exp)

                    # one-hot per row-tile: oh[p, k] = (label[p, t] == k)
                    # (single-source tensor_scalar -> fast DVE mode)
                    oh = ohpool.tile([P, HW], bf16)
                    for q in range(HPB):
                        t = h0 + q
                        nc.vector.tensor_scalar(
                            out=oh[:, q * K:(q + 1) * K], in0=iota[:],
                            scalar1=labc[:, t:t + 1], scalar2=None,
                            op0=mybir.AluOpType.is_equal)

                    # bin exp by label: S[c, k] += sum_p oh[p,c] * E[p,k]
                    for q in range(HPB):
                        t = h0 + q
                        b = t % NBANK
                        nc.tensor.matmul(
                            out=psums[b][:],
                            lhsT=oh[:, q * K:(q + 1) * K],
                            rhs=ee[:, q * K:(q + 1) * K],
                            start=(t < NBANK),
                            stop=(t >= NT - NBANK),
                        )

            osb = cpool.tile([P, NBANK * K], f32)
            for b in range(NBANK):
                if b % 2 == 0:
                    nc.vector.tensor_copy(osb[:, b * K:(b + 1) * K],
                                          psums[b][:])
                else:
                    nc.scalar.copy(osb[:, b * K:(b + 1) * K], psums[b][:])
            nc.sync.dma_start(out=out.ap(), in_=osb[:])

    nc.compile()
    return nc


def _shard_inputs(logits, labels):
    """Build the 8 per-core input maps (host-side layout only)."""
    logits = np.ascontiguousarray(np.asarray(logits, dtype=np.float32))
    labels = np.asarray(labels, dtype=np.int32)
    iota = np.broadcast_to(np.arange(K, dtype=np.float32), (P, K)).astype(
        ml_dtypes.bfloat16)
    in_maps = []
    for i in range(NCORES):
        sl = slice(i * BC, (i + 1) * BC)
        lab = labels[sl].astype(np.float32).reshape(P, NT)
        in_maps.append({
            "logits": logits[sl],
            "labcols": np.ascontiguousarray(lab),
            "iotak": iota,
        })
    return in_maps


def _finish(outs, logits, labels, events):
    """Host epilogue: all-reduce binned sums, triangular sum, numer/n_ev
    from 1-D data, the log, and the final scalar reduction."""
    labels = np.asarray(labels, dtype=np.int32)
    events = np.asarray(events, dtype=np.int32)
    S = np.zeros((P, K), dtype=np.float64)
    for o in outs:
        S += o.astype(np.float64).reshape(P, NBANK, K).sum(axis=1)
    # sumexp[k] = sum over label bins c >= k
    sumexp = (S * np.tri(K)).sum(axis=0)
    ev = events == 1
    own = np.asarray(logits)[np.arange(labels.shape[0]), labels].astype(
        np.float64)
    n_ev = np.bincount(labels[ev], minlength=K).astype(np.float64)
    numer = np.bincount(labels[ev], weights=own[ev], minlength=K)
    with np.errstate(divide="ignore"):
        denom_log = np.log(sumexp)
    terms = np.where(n_ev > 0, numer - n_ev * denom_log, 0.0)
    n_total = max(n_ev.sum(), 1.0)
    return np.array(-terms.sum() / n_total, dtype=np.float32)


def kernel(logits, labels, events, _trace=False):
    global LAST_EXEC_NS, LAST_TRACE, LAST_PROFILE_JSON
    in_maps = _shard_inputs(logits, labels)
    nc = build_nc()
    try:
        res = run_bass_kernel_spmd(nc, in_maps, core_ids=list(range(NCORES)),
                                   trace=_trace)
    except Exception:
        # one retry: absorbs transient NRT device-unrecoverable hiccups
        res = run_bass_kernel_spmd(nc, in_maps, core_ids=list(range(NCORES)),
                                   trace=_trace)
    LAST_EXEC_NS = res.exec_time_ns
    LAST_TRACE = res.instructions_and_trace
    LAST_PROFILE_JSON = res.profile_json
    outs = [res.results[i]["out"] for i in range(NCORES)]
    return _finish(outs, logits, labels, events)
